# revision 10
# baseline (speedup 1.0000x reference)
"""Multi-head attention forward (B=4, L=2048, E=1024, H=16) on 8 NeuronCores.

Sharding: core c handles batch b = c // 2 and head-group g = c % 2 (8 heads,
512 embed dims). Each core computes its QKV projections, attention, and a
partial out-projection over its 512 contraction dims; the host sums the two
partials per batch and adds the bias.

All transposes and bf16 casts happen on the host: each core receives
xqT/xkT/xvT as [E, L] bf16, wqkvT as [E, 3*FG] bf16 (cols q|k|v) and
woutT as [FG, E] bf16.

Schedule: the ACT engine's exp stream (one [128, 512*GRP] activation per
score group) is the critical resource (~270us); every projection
(k/q/v/out) is broken into 8-matmul bursts injected between attention
groups so the tensor engine's spare capacity under the exp cadence is
used and ACT never idles at iteration boundaries. kproj is windowed so
the first exp fires as soon as the first xk/xq windows arrive from HBM.

Self-contained: only needs numpy + the concourse stack at /opt/trn_rl_repo.
"""

import os
import sys

import numpy as np

sys.path.insert(0, "/opt/trn_rl_repo")

import ml_dtypes  # noqa: E402

import concourse.bass as bass  # noqa: E402
import concourse.tile as tile  # noqa: E402
from concourse import bacc, mybir  # noqa: E402
from concourse import bass_utils  # noqa: E402

F32 = mybir.dt.float32
BF16 = mybir.dt.bfloat16
EXP = mybir.ActivationFunctionType.Exp
NP_BF16 = ml_dtypes.bfloat16

P = 128          # partitions
L = 2048         # sequence length
E = 1024         # embed dim
FG = 512         # per-core feature slice (8 heads x 64)
D = 64           # head dim
EC = E // P      # 8 e-chunks (contraction tiles for projections)
SC = L // P      # 16 s-chunks
LG = L // 512    # 4 q-windows of 512
FT = FG // P     # 4 head pairs
GRP = 3          # score psum banks per exp group
NU = 2 * SC      # 32 (sc, head) units per (p, lg)
NGRP = (NU + GRP - 1) // GRP  # 11 groups per iteration


def _build():
    nc = bacc.Bacc("TRN2", target_bir_lowering=False, debug=False, num_devices=8)

    xqT_d = nc.dram_tensor("xqT", [E, L], BF16, kind="ExternalInput")
    xkT_d = nc.dram_tensor("xkT", [E, L], BF16, kind="ExternalInput")
    xvT_d = nc.dram_tensor("xvT", [E, L], BF16, kind="ExternalInput")
    wqkvT_d = nc.dram_tensor("wqkvT", [E, 3 * FG], BF16, kind="ExternalInput")
    woutT_d = nc.dram_tensor("woutT", [FG, E], BF16, kind="ExternalInput")
    out_d = nc.dram_tensor("out", [L, E], BF16, kind="ExternalOutput")

    with tile.TileContext(nc) as tc:
        with (
            tc.tile_pool(name="const", bufs=1) as constp,
            tc.tile_pool(name="pers", bufs=1) as pers,
            tc.tile_pool(name="xin", bufs=1) as xin,
            tc.tile_pool(name="xv", bufs=2) as xvp,
            tc.tile_pool(name="xq", bufs=2) as xqp,
            tc.tile_pool(name="stage", bufs=2) as stage,
            tc.tile_pool(name="ps", bufs=2, space="PSUM") as psp,
            tc.tile_pool(name="psav", bufs=1, space="PSUM") as psav,
        ):
            # engine warm-ups (prime DVE cast path + preload the EXP table)
            warm32 = constp.tile([P, 16], F32, tag="warm32", name="warm32")
            nc.vector.memset(warm32[:], 0.0)
            warm16 = constp.tile([P, 16], BF16, tag="warm16", name="warm16")
            nc.vector.tensor_copy(warm16[:], warm32[:])
            warmE = constp.tile([P, 16], BF16, tag="warmE", name="warmE")
            nc.scalar.activation(warmE[:], warm32[:], EXP, scale=0.125)
            warmG = constp.tile([P, 16], F32, tag="warmG", name="warmG")
            nc.gpsimd.memset(warmG[:], 0.0)
            # reciprocal staging: rows 0/32 hold denominators per tail, the
            # rest stays 1.0 so the batched [64,512] reciprocal is stable
            rr = constp.tile([P, 512], F32, tag="rr", name="rr")
            nc.vector.memset(rr[:], 1.0)

            # persistent activations / weights
            kT = [pers.tile([P, L], BF16, tag=f"kT{p}", name=f"kT{p}")
                  for p in range(FT)]
            qT = [pers.tile([P, L], BF16, tag=f"qT{p}", name=f"qT{p}")
                  for p in range(FT)]
            avN = [pers.tile([P, L], BF16, tag=f"avN{p}", name=f"avN{p}")
                   for p in range(FT)]
            # AV stationary tiles: per s-chunk, 4 pairs x 256 cols:
            #   [v_h0(64) | ones(1) | junk(63)]  -> av rows 0:64, sum row 64
            #   [junk(32) | ones(1) | junk(31) | v_h1(64)] -> rows 64:128, sum row 32
            vst = [pers.tile([P, 1024], BF16, tag=f"vst{s}", name=f"vst{s}")
                   for s in range(SC)]
            wq_sb = [pers.tile([P, 3 * FG], BF16, tag=f"wq{ec}", name=f"wq{ec}")
                     for ec in range(EC)]
            wo_sb = [pers.tile([P, E], BF16, tag=f"wo{e}", name=f"wo{e}")
                     for e in range(FT)]
            xk = [xin.tile([P, L], BF16, tag=f"x{ec}", name=f"xk{ec}")
                  for ec in range(EC)]

            # ---- DMA helpers ----
            def dma_w_slice(qkv, p0, p1):
                # wqkvT columns [qkv*FG + p0*P, qkv*FG + p1*P) for all ec
                c0, c1 = qkv * FG + p0 * P, qkv * FG + p1 * P
                for ec in range(EC):
                    nc.sync.dma_start(wq_sb[ec][:, c0:c1],
                                      wqkvT_d.ap()[ec * P:(ec + 1) * P, c0:c1])

            def dma_xk_window(w):
                for ec in range(EC):
                    nc.sync.dma_start(
                        xk[ec][:, w * 512:(w + 1) * 512],
                        xkT_d.ap()[ec * P:(ec + 1) * P, w * 512:(w + 1) * 512])

            xqwin = {}

            def dma_xq_window(lg):
                tiles = [xqp.tile([P, 512], BF16, tag=f"q{ec}", name=f"xq{ec}")
                         for ec in range(EC)]
                for ec in range(EC):
                    nc.sync.dma_start(
                        tiles[ec][:],
                        xqT_d.ap()[ec * P:(ec + 1) * P, lg * 512:(lg + 1) * 512])
                xqwin[lg] = tiles

            xvw = {}

            def dma_xv_window(w):
                tiles = [xvp.tile([P, 512], BF16, tag=f"v{ec}", name=f"xv{ec}")
                         for ec in range(EC)]
                for ec in range(EC):
                    nc.sync.dma_start(
                        tiles[ec][:],
                        xvT_d.ap()[ec * P:(ec + 1) * P, w * 512:(w + 1) * 512])
                xvw[w] = tiles

            # ---- prolog DMAs, in deadline order (one HBM pipe, ~2.8us/MB) ----
            dma_w_slice(1, 0, 1)       # wk slice for p=0
            dma_xk_window(0)
            dma_w_slice(2, 0, 4)       # full wv
            dma_xv_window(0)
            dma_w_slice(0, 0, 1)       # wq slice for p=0
            dma_xq_window(0)
            dma_xv_window(1)
            dma_xk_window(1)
            dma_w_slice(1, 1, 4)       # rest of wk
            dma_xv_window(2)
            dma_xk_window(2)
            dma_xv_window(3)
            dma_xk_window(3)
            dma_w_slice(0, 1, 4)       # rest of wq
            for e in range(FT):
                nc.sync.dma_start(wo_sb[e][:], woutT_d.ap()[e * P:(e + 1) * P, :])

            # AV stationary pattern: zero the non-v columns, ones at the
            # denominator columns (64, 160 of each 256-block)
            one = int(np.float32(1.0).astype(NP_BF16).view(np.uint16))
            for s in range(SC):
                t = vst[s]
                nc.gpsimd._memset_packed(
                    bass.AP(t.tensor, t.offset + 64,
                            [[1024, 128], [256, 4], [1, 128]]), 0)
                nc.gpsimd._memset_packed(
                    bass.AP(t.tensor, t.offset + 64, [[1024, 128], [256, 4]]),
                    one)
                nc.gpsimd._memset_packed(
                    bass.AP(t.tensor, t.offset + 160, [[1024, 128], [256, 4]]),
                    one)

            # ---- projection bursts ----
            def kproj_chunk(p, w):
                ps = psp.tile([P, 512 * GRP], F32, tag="sc", name="kps")
                for ec in range(EC):
                    nc.tensor.matmul(
                        ps[:, 0:512],
                        wq_sb[ec][:, FG + p * P:FG + (p + 1) * P],
                        xk[ec][:, w * 512:(w + 1) * 512],
                        start=(ec == 0), stop=(ec == EC - 1))
                nc.vector.tensor_copy(
                    kT[p][:, w * 512:(w + 1) * 512], ps[:, 0:512])

            def qproj(p, lg):
                tiles = xqwin[lg]
                ps = psp.tile([P, 512 * GRP], F32, tag="sc", name="qps")
                for ec in range(EC):
                    nc.tensor.matmul(
                        ps[:, 0:512],
                        wq_sb[ec][:, p * P:(p + 1) * P],
                        tiles[ec][:],
                        start=(ec == 0), stop=(ec == EC - 1))
                nc.vector.tensor_copy(
                    qT[p][:, lg * 512:(lg + 1) * 512], ps[:, 0:512])

            def vproj(lt):
                xv_t = xvw[lt // 4]
                ps = psp.tile([P, 512 * GRP], F32, tag="sc", name="vps")
                for ec in range(EC):
                    nc.tensor.matmul(
                        ps[:, 0:512],
                        xv_t[ec][:, (lt % 4) * P:(lt % 4 + 1) * P],
                        wq_sb[ec][:, 2 * FG:3 * FG],
                        start=(ec == 0), stop=(ec == EC - 1))
                # strided drains: h0 dims -> cols {0:64}+256p, h1 -> {192:256}+256p
                psw = 512 * GRP
                dst0 = bass.AP(vst[lt].tensor, vst[lt].offset,
                               [[1024, 128], [256, 4], [1, 64]])
                src0 = bass.AP(ps.tensor, ps.offset,
                               [[psw, 128], [128, 4], [1, 64]])
                nc.vector.tensor_copy(dst0, src0)
                dst1 = bass.AP(vst[lt].tensor, vst[lt].offset + 192,
                               [[1024, 128], [256, 4], [1, 64]])
                src1 = bass.AP(ps.tensor, ps.offset + 64,
                               [[psw, 128], [128, 4], [1, 64]])
                nc.vector.tensor_copy(dst1, src1)

            def outproj(lg, lt):
                t0 = lg * 512 + lt * P
                ps = psp.tile([P, 512 * GRP], F32, tag="sc", name="ops")
                for ec in range(FT):
                    nc.tensor.matmul(
                        ps[:, 0:512], avN[ec][:, t0:t0 + P],
                        wo_sb[ec][:, 0:512],
                        start=(ec == 0), stop=(ec == FT - 1))
                    nc.tensor.matmul(
                        ps[:, 512:1024], avN[ec][:, t0:t0 + P],
                        wo_sb[ec][:, 512:1024],
                        start=(ec == 0), stop=(ec == FT - 1))
                osb = stage.tile([P, E], BF16, tag="osb", name="osb", bufs=2)
                nc.vector.tensor_copy(osb[:], ps[:, 0:1024])
                nc.sync.dma_start(out_d.ap()[t0:t0 + P, :], osb[:])

            pending_tails = []

            def attention_iter(p, lg, sched):
                avA = psav.tile([P, 512], F32, tag="avA", name="avA")
                avB = psav.tile([P, 512], F32, tag="avB", name="avB")
                av_bank = (avA, avB)

                def av_mms(t0, n, aT, p=p, av_bank=av_bank):
                    for j in range(n):
                        sc, h = divmod(t0 + j, 2)
                        nc.tensor.matmul(
                            av_bank[h][:],
                            vst[sc][:, p * 256 + 128 * h:p * 256 + 128 * h + 128],
                            aT[:, j * 512:(j + 1) * 512],
                            start=(sc == 0), stop=(sc == SC - 1))

                pending = None
                for gi, t0 in enumerate(range(0, NU, GRP)):
                    n = min(GRP, NU - t0)
                    ps = psp.tile([P, 512 * GRP], F32, tag="sc", name="scp")
                    for j in range(n):
                        sc, h = divmod(t0 + j, 2)
                        nc.tensor.matmul(
                            ps[:, j * 512:(j + 1) * 512],
                            kT[p][64 * h:64 * h + 64, sc * P:(sc + 1) * P],
                            qT[p][64 * h:64 * h + 64, lg * 512:(lg + 1) * 512],
                            start=True, stop=True)
                    aT = stage.tile([P, 512 * GRP], BF16, tag="aT", name="aT",
                                    bufs=4)
                    nc.scalar.activation(aT[:, 0:512 * n], ps[:, 0:512 * n],
                                         EXP, scale=0.125)
                    if gi == 0 and pending_tails:
                        pending_tails.pop()()  # prev iter's tail after this
                        # iter's first exp is queued
                    for fn in sched.get(gi, ()):
                        fn()
                    if pending is not None:
                        av_mms(*pending)
                    pending = (t0, n, aT)

                def tail(avA=avA, avB=avB, p=p, lg=lg, pending=pending,
                         av_mms=av_mms):
                    av_mms(*pending)
                    avS0 = stage.tile([P, 512], F32, tag="avS0", name="avS0",
                                      bufs=1)
                    nc.vector.tensor_copy(avS0[:], avA[:])
                    avS1 = stage.tile([P, 512], F32, tag="avS1", name="avS1",
                                      bufs=1)
                    nc.vector.tensor_copy(avS1[:], avB[:])
                    # both denominators on partitions 0/32 -> one reciprocal
                    nc.vector.tensor_copy(rr[0:1, :], avS0[64:65, :])
                    nc.vector.tensor_copy(rr[32:33, :], avS1[32:33, :])
                    nc.vector.reciprocal(rr[0:64, :], rr[0:64, :])
                    r1 = stage.tile([1, 512], F32, tag="r1", name="r1", bufs=1)
                    nc.vector.tensor_copy(r1[0:1, :], rr[32:33, :])
                    bc0 = stage.tile([P, 512], F32, tag="bc0", name="bc0",
                                     bufs=1)
                    nc.gpsimd.partition_broadcast(bc0[:], rr[0:1, :])
                    bc1 = stage.tile([P, 512], F32, tag="bc1", name="bc1",
                                     bufs=1)
                    nc.gpsimd.partition_broadcast(bc1[:], r1[0:1, :])
                    nc.vector.tensor_mul(
                        avN[p][0:64, lg * 512:(lg + 1) * 512],
                        avS0[0:64, :], bc0[0:64, :])
                    nc.vector.tensor_mul(
                        avN[p][64:128, lg * 512:(lg + 1) * 512],
                        avS1[64:128, :], bc1[64:128, :])
                pending_tails.append(tail)

            # ---- prolog compute (overlapped with the DMA stream) ----
            kproj_chunk(0, 0)
            vproj(0)
            vproj(1)
            qproj(0, 0)

            # ---- the fused schedule ----
            def make_sched(lg, p):
                sched = {}
                if lg == 0 and p == 0:
                    sched[0] = [lambda: vproj(2), lambda: vproj(3)]
                    sched[1] = [lambda: vproj(4), lambda: kproj_chunk(0, 1)]
                    sched[2] = [lambda: vproj(5), lambda: vproj(6)]
                    sched[3] = [lambda: vproj(7)]
                    sched[4] = [lambda: kproj_chunk(0, 2), lambda: vproj(8)]
                    sched[5] = [lambda: vproj(9), lambda: vproj(10)]
                    sched[6] = [lambda: kproj_chunk(0, 3), lambda: vproj(11)]
                    sched[7] = [lambda: kproj_chunk(1, 0), lambda: vproj(12)]
                    sched[8] = [lambda: kproj_chunk(1, 1), lambda: vproj(13)]
                    sched[9] = [lambda: kproj_chunk(1, 2), lambda: qproj(1, 0),
                                lambda: vproj(14)]
                    sched[10] = [lambda: kproj_chunk(1, 3), lambda: vproj(15)]
                    return sched
                if lg == 0 and p == 1:
                    for i in range(4):
                        sched[2 * i] = [lambda i=i: kproj_chunk(2, i)]
                    sched[8] = [lambda: qproj(2, 0)]
                    return sched
                if lg == 0 and p == 2:
                    for i in range(4):
                        sched[2 * i] = [lambda i=i: kproj_chunk(3, i)]
                    sched[8] = [lambda: qproj(3, 0)]
                    return sched
                # steady state
                if lg >= 1:
                    sched[5] = [lambda: outproj(lg - 1, p)]
                if p < 3:
                    sched[8] = sched.get(8, []) + [lambda: qproj(p + 1, lg)]
                elif lg < 3:
                    sched[2] = [lambda: dma_xq_window(lg + 1)]
                    sched[8] = sched.get(8, []) + [lambda: qproj(0, lg + 1)]
                return sched

            for lg in range(LG):
                for p in range(FT):
                    attention_iter(p, lg, make_sched(lg, p))
            while pending_tails:
                pending_tails.pop()()
            for lt in range(FT):
                outproj(3, lt)

    nc.compile()
    return nc


_NC = None


def _get_nc():
    global _NC
    if _NC is None:
        _NC = _build()
    return _NC


def _shard_inputs(query, key, value, in_proj_weight, out_proj_weight):
    B = query.shape[0]
    # per-batch transposed bf16 activations (shared by the 2 cores per batch)
    xT = {}
    for b in range(B):
        xT[b] = tuple(
            np.ascontiguousarray(np.asarray(x[b], dtype=np.float32).T).astype(NP_BF16)
            for x in (query, key, value))
    # per-head-group weight blocks
    wblk = {}
    for g in range(2):
        sl = slice(FG * g, FG * g + FG)
        wq = in_proj_weight[0 * E:1 * E][sl]
        wk = in_proj_weight[1 * E:2 * E][sl]
        wv = in_proj_weight[2 * E:3 * E][sl]
        wqkvT = np.ascontiguousarray(
            np.concatenate([wq.T, wk.T, wv.T], axis=1)).astype(NP_BF16)
        woutT = np.ascontiguousarray(out_proj_weight[:, sl].T).astype(NP_BF16)
        wblk[g] = (wqkvT, woutT)
    in_maps = []
    for c in range(8):
        b, g = divmod(c, 2)
        xq, xk, xv = xT[b]
        wqkvT, woutT = wblk[g]
        in_maps.append({
            "xqT": xq, "xkT": xk, "xvT": xv,
            "wqkvT": wqkvT, "woutT": woutT,
        })
    return in_maps


def run_sharded(in_maps, **kwargs):
    nc = _get_nc()
    return bass_utils.run_bass_kernel_spmd(
        nc, in_maps, core_ids=list(range(8)), **kwargs)


def kernel(query, key, value, in_proj_weight, out_proj_weight, out_proj_bias):
    query = np.asarray(query, dtype=np.float32)
    key = np.asarray(key, dtype=np.float32)
    value = np.asarray(value, dtype=np.float32)
    in_proj_weight = np.asarray(in_proj_weight, dtype=np.float32)
    out_proj_weight = np.asarray(out_proj_weight, dtype=np.float32)
    out_proj_bias = np.asarray(out_proj_bias, dtype=np.float32)

    in_maps = _shard_inputs(query, key, value, in_proj_weight, out_proj_weight)
    res = run_sharded(in_maps)
    out = np.empty((4, L, E), dtype=np.float32)
    for b in range(4):
        out[b] = (np.asarray(res.results[2 * b]["out"], dtype=np.float32)
                  + np.asarray(res.results[2 * b + 1]["out"], dtype=np.float32))
    out += out_proj_bias
    return out


# revision 11
# speedup vs baseline: 1.0271x; 1.0271x over previous
"""Multi-head attention forward (B=4, L=2048, E=1024, H=16) on 8 NeuronCores.

Sharding: core c handles batch b = c // 2 and head-group g = c % 2 (8 heads,
512 embed dims). Each core computes its QKV projections, attention, and a
partial out-projection over its 512 contraction dims; the host sums the two
partials per batch and adds the bias.

All transposes and bf16 casts happen on the host: each core receives
xqT/xkT/xvT as [E, L] bf16, wqkvT as [E, 3*FG] bf16 (cols q|k|v) and
woutT as [FG, E] bf16.

Schedule: the ACT engine's exp stream (one [128, 512*GRP] activation per
score group) is the critical resource (~270us); every projection
(k/q/v/out) is broken into 8/16-matmul bursts injected between attention
groups so the tensor engine's spare capacity under the exp cadence is
used and ACT never idles at iteration boundaries. All input staging uses
single multi-dim DMA descriptors (the sync queue issues ~0.6us per
descriptor, so per-chunk DMAs would gate the whole prolog).

Self-contained: only needs numpy + the concourse stack at /opt/trn_rl_repo.
"""

import sys

import numpy as np

sys.path.insert(0, "/opt/trn_rl_repo")

import ml_dtypes  # noqa: E402

import concourse.bass as bass  # noqa: E402
import concourse.tile as tile  # noqa: E402
from concourse import bacc, mybir  # noqa: E402
from concourse import bass_utils  # noqa: E402

F32 = mybir.dt.float32
BF16 = mybir.dt.bfloat16
EXP = mybir.ActivationFunctionType.Exp
NP_BF16 = ml_dtypes.bfloat16

P = 128          # partitions
L = 2048         # sequence length
E = 1024         # embed dim
FG = 512         # per-core feature slice (8 heads x 64)
D = 64           # head dim
EC = E // P      # 8 e-chunks (contraction tiles for projections)
SC = L // P      # 16 s-chunks
LG = L // 512    # 4 q-windows of 512
FT = FG // P     # 4 head pairs
GRP = 3          # score psum banks per exp group
NU = 2 * SC      # 32 (sc, head) units per (p, lg)
W3 = 3 * FG      # wqkv row width (1536)


def _build():
    nc = bacc.Bacc("TRN2", target_bir_lowering=False, debug=False, num_devices=8)

    xqT_d = nc.dram_tensor("xqT", [E, L], BF16, kind="ExternalInput")
    xkT_d = nc.dram_tensor("xkT", [E, L], BF16, kind="ExternalInput")
    xvT_d = nc.dram_tensor("xvT", [E, L], BF16, kind="ExternalInput")
    wqkvT_d = nc.dram_tensor("wqkvT", [E, W3], BF16, kind="ExternalInput")
    woutT_d = nc.dram_tensor("woutT", [FG, E], BF16, kind="ExternalInput")
    out_d = nc.dram_tensor("out", [L, E], BF16, kind="ExternalOutput")

    with tile.TileContext(nc) as tc:
        with (
            tc.tile_pool(name="const", bufs=1) as constp,
            tc.tile_pool(name="pers", bufs=1) as pers,
            tc.tile_pool(name="xin", bufs=1) as xin,
            tc.tile_pool(name="xv", bufs=2) as xvp,
            tc.tile_pool(name="xq", bufs=2) as xqp,
            tc.tile_pool(name="stage", bufs=2) as stage,
            tc.tile_pool(name="ps", bufs=2, space="PSUM") as psp,
            tc.tile_pool(name="psav", bufs=1, space="PSUM") as psav,
        ):
            # engine warm-ups (prime DVE cast path + preload the EXP table)
            warm32 = constp.tile([P, 16], F32, tag="warm32", name="warm32")
            nc.vector.memset(warm32[:], 0.0)
            warm16 = constp.tile([P, 16], BF16, tag="warm16", name="warm16")
            nc.vector.tensor_copy(warm16[:], warm32[:])
            warmE = constp.tile([P, 16], BF16, tag="warmE", name="warmE")
            nc.scalar.activation(warmE[:], warm32[:], EXP, scale=0.125)
            warmG = constp.tile([P, 16], F32, tag="warmG", name="warmG")
            nc.gpsimd.memset(warmG[:], 0.0)
            # reciprocal staging: rows 0/32 hold denominators per tail, the
            # rest stays 1.0 so the batched [64,512] reciprocal is stable
            rr = constp.tile([P, 512], F32, tag="rr", name="rr")
            nc.vector.memset(rr[:], 1.0)

            # persistent activations / weights (ec-major fused layouts)
            kT = [pers.tile([P, L], BF16, tag=f"kT{p}", name=f"kT{p}")
                  for p in range(FT)]
            qT = [pers.tile([P, L], BF16, tag=f"qT{p}", name=f"qT{p}")
                  for p in range(FT)]
            avN = [pers.tile([P, L], BF16, tag=f"avN{p}", name=f"avN{p}")
                   for p in range(FT)]
            # AV stationary tiles: per s-chunk, 4 pairs x 256 cols:
            #   [v_h0(64) | ones(1) | junk(63)]  -> av rows 0:64, sum row 64
            #   [junk(32) | ones(1) | junk(31) | v_h1(64)] -> rows 64:128, sum row 32
            vst = [pers.tile([P, 1024], BF16, tag=f"vst{s}", name=f"vst{s}")
                   for s in range(SC)]
            wqA = pers.tile([P, EC * W3], BF16, tag="wqA", name="wqA")
            woA = pers.tile([P, FT * E], BF16, tag="woA", name="woA")
            xkA = xin.tile([P, EC * L], BF16, tag="xkA", name="xkA")

            # ---- fused DMA helpers (one descriptor per logical transfer) ----
            def dma_w_slice(qkv, p0, p1):
                w = (p1 - p0) * P
                c0 = qkv * FG + p0 * P
                s = wqkvT_d.ap()
                nc.sync.dma_start(
                    bass.AP(wqA.tensor, wqA.offset + c0,
                            [[EC * W3, P], [W3, EC], [1, w]]),
                    bass.AP(s.tensor, s.offset + c0,
                            [[W3, P], [P * W3, EC], [1, w]]))

            def dma_xk_window(w):
                s = xkT_d.ap()
                nc.sync.dma_start(
                    bass.AP(xkA.tensor, xkA.offset + w * 512,
                            [[EC * L, P], [L, EC], [1, 512]]),
                    bass.AP(s.tensor, s.offset + w * 512,
                            [[L, P], [P * L, EC], [1, 512]]))

            def dma_wout():
                s = woutT_d.ap()
                nc.sync.dma_start(
                    bass.AP(woA.tensor, woA.offset,
                            [[FT * E, P], [E, FT], [1, E]]),
                    bass.AP(s.tensor, s.offset,
                            [[E, P], [P * E, FT], [1, E]]))

            xqwin = {}

            def dma_xq_window(lg):
                t = xqp.tile([P, EC * 512], BF16, tag="xqw", name="xqw")
                s = xqT_d.ap()
                nc.sync.dma_start(
                    bass.AP(t.tensor, t.offset,
                            [[EC * 512, P], [512, EC], [1, 512]]),
                    bass.AP(s.tensor, s.offset + lg * 512,
                            [[L, P], [P * L, EC], [1, 512]]))
                xqwin[lg] = t

            xvw = {}

            def dma_xv_window(w):
                t = xvp.tile([P, EC * 512], BF16, tag="xvw", name="xvw")
                s = xvT_d.ap()
                nc.sync.dma_start(
                    bass.AP(t.tensor, t.offset,
                            [[EC * 512, P], [512, EC], [1, 512]]),
                    bass.AP(s.tensor, s.offset + w * 512,
                            [[L, P], [P * L, EC], [1, 512]]))
                xvw[w] = t

            # ---- prolog DMAs, in deadline order (one HBM pipe, ~2.8us/MB) --
            dma_w_slice(1, 0, 1)       # wk slice for p=0
            dma_xk_window(0)
            dma_w_slice(2, 0, 4)       # full wv
            dma_xv_window(0)
            dma_w_slice(0, 0, 1)       # wq slice for p=0
            dma_xq_window(0)
            dma_xv_window(1)
            dma_xk_window(1)
            dma_w_slice(1, 1, 4)       # rest of wk
            dma_xv_window(2)
            dma_xk_window(2)
            dma_xv_window(3)
            dma_xk_window(3)
            dma_w_slice(0, 1, 4)       # rest of wq
            dma_wout()

            # AV stationary pattern: zero the non-v columns, ones at the
            # denominator columns (64, 160 of each 256-block)
            one = int(np.float32(1.0).astype(NP_BF16).view(np.uint16))
            for s in range(SC):
                t = vst[s]
                nc.gpsimd._memset_packed(
                    bass.AP(t.tensor, t.offset + 64,
                            [[1024, 128], [256, 4], [1, 128]]), 0)
                nc.gpsimd._memset_packed(
                    bass.AP(t.tensor, t.offset + 64, [[1024, 128], [256, 4]]),
                    one)
                nc.gpsimd._memset_packed(
                    bass.AP(t.tensor, t.offset + 160, [[1024, 128], [256, 4]]),
                    one)

            # ---- projection bursts ----
            def wq_cols(ec, c0, c1):
                return wqA[:, ec * W3 + c0:ec * W3 + c1]

            def kproj_into(p, w, ps, b):
                for ec in range(EC):
                    nc.tensor.matmul(
                        ps[:, b * 512:b * 512 + 512],
                        wq_cols(ec, FG + p * P, FG + (p + 1) * P),
                        xkA[:, ec * L + w * 512:ec * L + (w + 1) * 512],
                        start=(ec == 0), stop=(ec == EC - 1))
                nc.vector.tensor_copy(
                    kT[p][:, w * 512:(w + 1) * 512], ps[:, b * 512:b * 512 + 512])

            def kproj_chunk(p, w):
                ps = psp.tile([P, 512 * GRP], F32, tag="sc", name="kps")
                kproj_into(p, w, ps, 0)

            def kproj_pair(p, w0):
                ps = psp.tile([P, 512 * GRP], F32, tag="sc", name="kps")
                kproj_into(p, w0, ps, 0)
                kproj_into(p, w0 + 1, ps, 1)

            def qproj_into(p, lg, ps, b):
                t = xqwin[lg]
                for ec in range(EC):
                    nc.tensor.matmul(
                        ps[:, b * 512:b * 512 + 512],
                        wq_cols(ec, p * P, (p + 1) * P),
                        t[:, ec * 512:(ec + 1) * 512],
                        start=(ec == 0), stop=(ec == EC - 1))
                nc.vector.tensor_copy(
                    qT[p][:, lg * 512:(lg + 1) * 512], ps[:, b * 512:b * 512 + 512])

            def qproj(p, lg):
                ps = psp.tile([P, 512 * GRP], F32, tag="sc", name="qps")
                qproj_into(p, lg, ps, 0)

            def vproj_into(lt, ps, b):
                t = xvw[lt // 4]
                for ec in range(EC):
                    nc.tensor.matmul(
                        ps[:, b * 512:b * 512 + 512],
                        t[:, ec * 512 + (lt % 4) * P:ec * 512 + (lt % 4 + 1) * P],
                        wq_cols(ec, 2 * FG, 3 * FG),
                        start=(ec == 0), stop=(ec == EC - 1))
                # strided drains: h0 dims -> cols {0:64}+256p, h1 -> {192:256}+256p
                psw = 512 * GRP
                dst0 = bass.AP(vst[lt].tensor, vst[lt].offset,
                               [[1024, 128], [256, 4], [1, 64]])
                src0 = bass.AP(ps.tensor, ps.offset + b * 512,
                               [[psw, 128], [128, 4], [1, 64]])
                nc.vector.tensor_copy(dst0, src0)
                dst1 = bass.AP(vst[lt].tensor, vst[lt].offset + 192,
                               [[1024, 128], [256, 4], [1, 64]])
                src1 = bass.AP(ps.tensor, ps.offset + b * 512 + 64,
                               [[psw, 128], [128, 4], [1, 64]])
                nc.vector.tensor_copy(dst1, src1)

            def vproj_pair(lt0):
                ps = psp.tile([P, 512 * GRP], F32, tag="sc", name="vps")
                vproj_into(lt0, ps, 0)
                vproj_into(lt0 + 1, ps, 1)

            def outproj_into(lg, lt, ps, b):
                t0 = lg * 512 + lt * P
                for ec in range(FT):
                    nc.tensor.matmul(
                        ps[:, b * 512:b * 512 + 512], avN[ec][:, t0:t0 + P],
                        woA[:, ec * E:ec * E + 512],
                        start=(ec == 0), stop=(ec == FT - 1))
                    nc.tensor.matmul(
                        ps[:, b * 512 + 512:b * 512 + 1024],
                        avN[ec][:, t0:t0 + P],
                        woA[:, ec * E + 512:ec * E + 1024],
                        start=(ec == 0), stop=(ec == FT - 1))
                osb = stage.tile([P, E], BF16, tag="osb", name="osb", bufs=2)
                nc.vector.tensor_copy(osb[:], ps[:, b * 512:b * 512 + 1024])
                nc.sync.dma_start(out_d.ap()[t0:t0 + P, :], osb[:])

            def outproj(lg, lt):
                ps = psp.tile([P, 512 * GRP], F32, tag="sc", name="ops")
                outproj_into(lg, lt, ps, 0)

            def qout_burst(qp, qlg, olg, olt):
                # one psum tile: qproj in bank 0, outproj in banks 1-2
                ps = psp.tile([P, 512 * GRP], F32, tag="sc", name="qob")
                qproj_into(qp, qlg, ps, 0)
                outproj_into(olg, olt, ps, 1)

            pending_tails = []

            def attention_iter(p, lg, sched):
                avA = psav.tile([P, 512], F32, tag="avA", name="avA")
                avB = psav.tile([P, 512], F32, tag="avB", name="avB")
                av_bank = (avA, avB)

                def av_mms(t0, n, aT, p=p, av_bank=av_bank):
                    for j in range(n):
                        sc, h = divmod(t0 + j, 2)
                        nc.tensor.matmul(
                            av_bank[h][:],
                            vst[sc][:, p * 256 + 128 * h:p * 256 + 128 * h + 128],
                            aT[:, j * 512:(j + 1) * 512],
                            start=(sc == 0), stop=(sc == SC - 1))

                pending = None
                for gi, t0 in enumerate(range(0, NU, GRP)):
                    n = min(GRP, NU - t0)
                    ps = psp.tile([P, 512 * GRP], F32, tag="sc", name="scp")
                    for j in range(n):
                        sc, h = divmod(t0 + j, 2)
                        nc.tensor.matmul(
                            ps[:, j * 512:(j + 1) * 512],
                            kT[p][64 * h:64 * h + 64, sc * P:(sc + 1) * P],
                            qT[p][64 * h:64 * h + 64, lg * 512:(lg + 1) * 512],
                            start=True, stop=True)
                    aT = stage.tile([P, 512 * GRP], BF16, tag="aT", name="aT",
                                    bufs=4)
                    nc.scalar.activation(aT[:, 0:512 * n], ps[:, 0:512 * n],
                                         EXP, scale=0.125)
                    if gi == 0 and pending_tails:
                        pending_tails.pop()()  # prev iter's tail after this
                        # iter's first exp is queued
                    for fn in sched.get(gi, ()):
                        fn()
                    if pending is not None:
                        av_mms(*pending)
                    pending = (t0, n, aT)

                def tail(avA=avA, avB=avB, p=p, lg=lg, pending=pending,
                         av_mms=av_mms):
                    av_mms(*pending)
                    avS0 = stage.tile([P, 512], F32, tag="avS0", name="avS0",
                                      bufs=1)
                    nc.vector.tensor_copy(avS0[:], avA[:])
                    avS1 = stage.tile([P, 512], F32, tag="avS1", name="avS1",
                                      bufs=1)
                    nc.vector.tensor_copy(avS1[:], avB[:])
                    # both denominators on partitions 0/32 -> one reciprocal
                    nc.vector.tensor_copy(rr[0:1, :], avS0[64:65, :])
                    nc.vector.tensor_copy(rr[32:33, :], avS1[32:33, :])
                    nc.vector.reciprocal(rr[0:64, :], rr[0:64, :])
                    r1 = stage.tile([1, 512], F32, tag="r1", name="r1", bufs=1)
                    nc.vector.tensor_copy(r1[0:1, :], rr[32:33, :])
                    bc0 = stage.tile([P, 512], F32, tag="bc0", name="bc0",
                                     bufs=1)
                    nc.gpsimd.partition_broadcast(bc0[:], rr[0:1, :])
                    bc1 = stage.tile([P, 512], F32, tag="bc1", name="bc1",
                                     bufs=1)
                    nc.gpsimd.partition_broadcast(bc1[:], r1[0:1, :])
                    nc.vector.tensor_mul(
                        avN[p][0:64, lg * 512:(lg + 1) * 512],
                        avS0[0:64, :], bc0[0:64, :])
                    nc.vector.tensor_mul(
                        avN[p][64:128, lg * 512:(lg + 1) * 512],
                        avS1[64:128, :], bc1[64:128, :])
                pending_tails.append(tail)

            # ---- prolog compute (overlapped with the DMA stream) ----
            kproj_chunk(0, 0)
            vproj_pair(0)
            qproj(0, 0)

            # ---- the fused schedule ----
            def make_sched(lg, p):
                sched = {}
                if lg == 0 and p == 0:
                    sched[0] = [lambda: vproj_pair(2)]
                    sched[1] = [lambda: kproj_chunk(0, 1),
                                lambda: vproj_pair(4)]
                    sched[2] = [lambda: vproj_pair(6)]
                    sched[4] = [lambda: kproj_chunk(0, 2),
                                lambda: vproj_pair(8)]
                    sched[5] = [lambda: vproj_pair(10)]
                    sched[6] = [lambda: kproj_chunk(0, 3)]
                    sched[7] = [lambda: kproj_pair(1, 0),
                                lambda: vproj_pair(12)]
                    sched[8] = [lambda: kproj_pair(1, 2)]
                    sched[9] = [lambda: qproj(1, 0), lambda: vproj_pair(14)]
                    return sched
                if lg == 0 and p == 1:
                    sched[0] = [lambda: kproj_pair(2, 0)]
                    sched[4] = [lambda: kproj_pair(2, 2)]
                    sched[8] = [lambda: qproj(2, 0)]
                    return sched
                if lg == 0 and p == 2:
                    sched[0] = [lambda: kproj_pair(3, 0)]
                    sched[4] = [lambda: kproj_pair(3, 2)]
                    sched[8] = [lambda: qproj(3, 0)]
                    return sched
                if lg == 0 and p == 3:
                    sched[2] = [lambda: dma_xq_window(1)]
                    sched[8] = [lambda: qproj(0, 1)]
                    return sched
                # steady state (lg >= 1): merge qproj-next + outproj-prev
                if p < 3:
                    sched[5] = [lambda: qout_burst(p + 1, lg, lg - 1, p)]
                elif lg < 3:
                    sched[2] = [lambda: dma_xq_window(lg + 1)]
                    sched[5] = [lambda: qout_burst(0, lg + 1, lg - 1, p)]
                else:
                    sched[5] = [lambda: outproj(lg - 1, p)]
                return sched

            for lg in range(LG):
                for p in range(FT):
                    attention_iter(p, lg, make_sched(lg, p))
            while pending_tails:
                pending_tails.pop()()
            for lt in range(FT):
                outproj(3, lt)

    nc.compile()
    return nc


_NC = None


def _get_nc():
    global _NC
    if _NC is None:
        _NC = _build()
    return _NC


def _shard_inputs(query, key, value, in_proj_weight, out_proj_weight):
    B = query.shape[0]
    # per-batch transposed bf16 activations (shared by the 2 cores per batch)
    xT = {}
    for b in range(B):
        xT[b] = tuple(
            np.ascontiguousarray(np.asarray(x[b], dtype=np.float32).T).astype(NP_BF16)
            for x in (query, key, value))
    # per-head-group weight blocks
    wblk = {}
    for g in range(2):
        sl = slice(FG * g, FG * g + FG)
        wq = in_proj_weight[0 * E:1 * E][sl]
        wk = in_proj_weight[1 * E:2 * E][sl]
        wv = in_proj_weight[2 * E:3 * E][sl]
        wqkvT = np.ascontiguousarray(
            np.concatenate([wq.T, wk.T, wv.T], axis=1)).astype(NP_BF16)
        woutT = np.ascontiguousarray(out_proj_weight[:, sl].T).astype(NP_BF16)
        wblk[g] = (wqkvT, woutT)
    in_maps = []
    for c in range(8):
        b, g = divmod(c, 2)
        xq, xk, xv = xT[b]
        wqkvT, woutT = wblk[g]
        in_maps.append({
            "xqT": xq, "xkT": xk, "xvT": xv,
            "wqkvT": wqkvT, "woutT": woutT,
        })
    return in_maps


def run_sharded(in_maps, **kwargs):
    nc = _get_nc()
    return bass_utils.run_bass_kernel_spmd(
        nc, in_maps, core_ids=list(range(8)), **kwargs)


def kernel(query, key, value, in_proj_weight, out_proj_weight, out_proj_bias):
    query = np.asarray(query, dtype=np.float32)
    key = np.asarray(key, dtype=np.float32)
    value = np.asarray(value, dtype=np.float32)
    in_proj_weight = np.asarray(in_proj_weight, dtype=np.float32)
    out_proj_weight = np.asarray(out_proj_weight, dtype=np.float32)
    out_proj_bias = np.asarray(out_proj_bias, dtype=np.float32)

    in_maps = _shard_inputs(query, key, value, in_proj_weight, out_proj_weight)
    res = run_sharded(in_maps)
    out = np.empty((4, L, E), dtype=np.float32)
    for b in range(4):
        out[b] = (np.asarray(res.results[2 * b]["out"], dtype=np.float32)
                  + np.asarray(res.results[2 * b + 1]["out"], dtype=np.float32))
    out += out_proj_bias
    return out
